# revision 35
# baseline (speedup 1.0000x reference)
"""GAT (2-layer graph attention network) Trainium2 kernel.

Contract: kernel(**inputs) takes the FULL inputs from setup_inputs() and
returns the full (32, 256, 512) float32 output. Internally shards the batch
across 8 NeuronCores (4 graphs per core), runs a Bass/Tile kernel per core,
and concatenates the results.

Math notes (reference in reference.py):
  x = embed[fea]                       -> computed on device as onehot @ embed
  Layer1 per head h: Wh = x @ W[h];  e1 = Wh @ a1 = x @ (W[h] @ a1)
    so [Wh | e1] come from ONE matmul with rhs [W[h] | W@a1]; the e2 vectors
    for ALL heads come from one skinny matmul (W@a2 stacked, M=8) against xT,
    then are broadcast across partitions with a ones(1,128) matmul into PSUM.
  e = leaky_relu(e1[:,None] + e2[None,:], 0.2); mask; softmax; out = attn@Wh.
  Softmax skips max-subtraction: e is O(1) for real rows; masked entries get
  -1000 added pre-leaky so exp(0.2*(-1000+e)) underflows to 0, matching
  where(mask, e, -9e15).
  Layer1 output is produced o-major (Wh.T @ pT), which is exactly the hT
  layout layer 2 needs for its stationary operand - no h transpose.
  non_pad_mask folds into layer-2's PSUM->SBUF copy scale and the final
  activation scale (multiplying rows of a matmul output == masking its input).
"""

import numpy as np
from contextlib import ExitStack

import concourse.bass as bass
import concourse.tile as tile
from concourse import mybir, bacc
from concourse.bass_utils import run_bass_kernel_spmd

f32 = mybir.dt.float32
i32 = mybir.dt.int32
AF = mybir.ActivationFunctionType
AL = mybir.AluOpType

# Problem dims (hardcoded per contract)
B, N, VOCAB, F, O, H, OUT = 32, 256, 200, 300, 256, 8, 512
NCORES = 8
GPC = B // NCORES          # graphs per core
NC = N // 128              # node chunks (2)
FC = 3                     # feature chunks (F padded 300->384)
VC = 2                     # vocab chunks (padded 200->256)
KC2 = (H * O) // 128       # layer-2 contraction chunks (16)
ALPHA = 0.2
MASK_NEG = -1000.0

# Matmul dtype: float32 (exact, 4 cyc/row) or float32r (tf32-like, 1 cyc/row)
MM_DT = mybir.dt.float32r


def _build_nc(mm_dt):
    nc = bacc.Bacc("TRN2", target_bir_lowering=False, debug=False,
                   num_devices=NCORES)

    oh_d = nc.dram_tensor("oh", [GPC, 128, VC, N], f32, kind="ExternalInput").ap()
    adj_d = nc.dram_tensor("adjm", [GPC, 128, NC, N], i32, kind="ExternalInput").ap()
    npm_d = nc.dram_tensor("npm", [GPC, 128, NC], f32, kind="ExternalInput").ap()
    ew_d = nc.dram_tensor("embw", [128, H, VC, O + 2], f32, kind="ExternalInput").ap()
    e2w_d = nc.dram_tensor("e2w", [128, VC, H], f32, kind="ExternalInput").ap()
    wo_d = nc.dram_tensor("woaug", [128, KC2, 2, 258], f32, kind="ExternalInput").ap()
    idn_d = nc.dram_tensor("identity", [128, 128], f32, kind="ExternalInput").ap()
    out_d = nc.dram_tensor("out", [GPC, 128, NC, OUT], f32, kind="ExternalOutput").ap()

    with tile.TileContext(nc) as tc, ExitStack() as ctx:
        const = ctx.enter_context(tc.tile_pool(name="const", bufs=1))
        gpool = ctx.enter_context(tc.tile_pool(name="gpool", bufs=2))
        hpool = ctx.enter_context(tc.tile_pool(name="hpool", bufs=4))
        hbig = ctx.enter_context(tc.tile_pool(name="hbig", bufs=2))
        ps_aug = ctx.enter_context(tc.tile_pool(name="ps_aug", bufs=2, space="PSUM"))
        ps_big = ctx.enter_context(tc.tile_pool(name="ps_big", bufs=3, space="PSUM"))
        ps_tr = ctx.enter_context(tc.tile_pool(name="ps_tr", bufs=1, space="PSUM"))
        ps_bc = ctx.enter_context(tc.tile_pool(name="ps_bc", bufs=2, space="PSUM"))

        # ---- resident constants ----
        ident = const.tile([128, 128], f32)
        nc.sync.dma_start(ident[:], idn_d)
        ones_f = const.tile([1, 128], f32)
        nc.vector.memset(ones_f[:], 1.0)

        stage = ctx.enter_context(tc.tile_pool(name="stage", bufs=2))

        def staged(dst_ap, src_ap, nfree):
            # stage f32 DMA -> compute-copy so the tile is a "rounded" f32r
            # producer; dst/src must be 2D-viewable as (128, nfree)
            if mm_dt == f32:
                nc.sync.dma_start(dst_ap, src_ap)
            else:
                st = stage.tile([128, 900], f32, tag="st")
                nc.sync.dma_start(st[:, :nfree], src_ap)
                nc.gpsimd.tensor_copy(dst_ap, st[:, :nfree])

        ew_sb = const.tile([128, H, VC, O + 2], mm_dt)
        for h in range(H):
            staged(ew_sb[:, h].rearrange("p a b -> p (a b)"),
                   ew_d[:, h].rearrange("p a b -> p (a b)"), VC * (O + 2))
        e2w_sb = const.tile([128, VC, H], mm_dt)
        staged(e2w_sb[:].rearrange("p a b -> p (a b)"),
               e2w_d[:].rearrange("p a b -> p (a b)"), VC * H)
        wo_sb = const.tile([128, KC2, 2, 258], mm_dt)
        for k in range(KC2):
            staged(wo_sb[:, k].rearrange("p a b -> p (a b)"),
                   wo_d[:, k].rearrange("p a b -> p (a b)"), 2 * 258)
        ones_sb = const.tile([1, 128], mm_dt)
        nc.vector.tensor_copy(ones_sb[:], ones_f[:])
        ones3_f = const.tile([65, 128], f32)
        nc.vector.memset(ones3_f[:], 1.0)
        ones3 = const.tile([65, 128], mm_dt)
        nc.vector.tensor_copy(ones3[:], ones3_f[:])
        ident_r = const.tile([128, 128], mm_dt)
        nc.vector.tensor_copy(ident_r[:], ident[:])

        # per-graph emission, interleaved in pairs: two independent graphs'
        # chains give every engine ready work while the other graph's
        # serial softmax/attention chain is in flight.
        G = {}

        def emit_setup(g):
            s = G[g] = {}
            oh_f = gpool.tile([128, VC, N], f32)
            nc.sync.dma_start(oh_f[:], oh_d[g])
            if mm_dt == f32:
                oh_sb = oh_f
            else:
                oh_sb = gpool.tile([128, VC, N], mm_dt)
                nc.vector.tensor_copy(oh_sb[:], oh_f[:])
            adj_sb = gpool.tile([128, NC, N], i32)
            nc.sync.dma_start(adj_sb[:], adj_d[g])
            npm_sb = gpool.tile([128, NC], f32)
            nc.sync.dma_start(npm_sb[:], npm_d[g])
            # mneg[n, m] = 0 where edge, MASK_NEG where not (exact in f32r)
            mneg = gpool.tile([128, NC, N], mm_dt)
            nc.scalar.activation(mneg[:], adj_sb[:], AF.Copy,
                                 bias=MASK_NEG, scale=-MASK_NEG)
            # e2 rows for all heads: (8, N) = (embed@w2).T @ onehot
            e2ps = ps_aug.tile([8, N], f32, tag="aug")
            for vc in range(VC):
                nc.tensor.matmul(e2ps[:], lhsT=e2w_sb[:, vc, :],
                                 rhs=oh_sb[:, vc, :],
                                 start=(vc == 0), stop=(vc == VC - 1))
            e2all = gpool.tile([8, N], f32)
            nc.vector.tensor_copy(e2all[:], e2ps[:])
            # group head rows onto base partitions 0/32/64 (3 per group)
            e2rs_f = gpool.tile([65, 3, N], f32)
            for i in range(3):
                nh = min(3, H - 3 * i)
                nc.scalar.dma_start(e2rs_f[32 * i:32 * i + 1, 0:nh, :],
                                     e2all[3 * i:3 * i + nh, :])
            e2rs = gpool.tile([65, 3, N], mm_dt)
            nc.vector.tensor_copy(e2rs[:], e2rs_f[:])
            hT = hbig.tile([128, KC2, N], mm_dt)
            s.update(oh_sb=oh_sb, npm=npm_sb, mneg=mneg, e2rs=e2rs,
                     hT=hT, wh={})

        def emit_aug(g, h):
            s = G[g]
            wh_sb = hpool.tile([128, NC, O + 2], mm_dt, tag="wh_sb")
            s["wh"][h] = wh_sb
            for c in range(NC):
                aug = ps_aug.tile([128, O + 2], f32, tag="aug")
                for vc in range(VC):
                    nc.tensor.matmul(
                        aug[:], lhsT=s["oh_sb"][:, vc, c * 128:(c + 1) * 128],
                        rhs=ew_sb[:, h, vc, :],
                        start=(vc == 0), stop=(vc == VC - 1))
                nc.scalar.copy(wh_sb[:, c, :], aug[:])

        def emit_head(g, h):
            s = G[g]
            wh_sb = s["wh"].pop(h)
            mneg, hT = s["mneg"], s["hT"]
            # broadcast head's e2 row into PSUM
            gi, gj = h // 3, h % 3
            e2bc = ps_bc.tile([128, N], f32, tag="bc")
            nc.tensor.matmul(e2bc[:],
                             lhsT=ones3[32 * gi:32 * gi + 1, :],
                             rhs=s["e2rs"][32 * gi:32 * gi + 1, gj, :],
                             start=True, stop=True)
            # softmax(leaky(e1 + e2 + mask))
            em = hpool.tile([128, NC, N], f32)
            zt = hpool.tile([128, NC, N], mm_dt)
            zsum = hpool.tile([128, NC], f32)
            zinv = hpool.tile([128, NC], f32)
            bca = e2bc[:]
            e2bc_b = bass.AP(tensor=bca.tensor, offset=bca.offset,
                             ap=[bca.ap[0], [0, NC], [1, N]])
            nc.vector.tensor_tensor(em[:], e2bc_b, mneg[:], op=AL.add)
            for c in range(NC):
                e1col = wh_sb[:, c, O:O + 1].bitcast(f32)
                nc.vector.tensor_scalar(
                    zt[:, c, :], em[:, c, :], e1col, None, op0=AL.add)
            nc.vector.tensor_scalar_mul(em[:], zt[:], ALPHA)
            nc.vector.tensor_tensor(zt[:], zt[:], em[:], op=AL.max)
            p_sb = zt[:]
            for c in range(NC):
                nc.scalar.activation(
                    p_sb[:, c, :], zt[:, c, :], AF.Exp,
                    accum_out=zsum[:, c:c + 1])
            nc.vector.reciprocal(zinv[:], zsum[:])
            for c in range(NC):
                nc.vector.tensor_scalar_mul(
                    p_sb[:, c, :], p_sb[:, c, :], zinv[:, c:c + 1])
            # transpose p into one PSUM bank, single copy out
            tp4 = ps_tr.tile([128, NC * NC, 128], mm_dt, tag="tr")
            for c in range(NC):
                for d in range(NC):
                    nc.tensor.transpose(
                        tp4[:, c * NC + d, :],
                        p_sb[:, c, d * 128:(d + 1) * 128],
                        ident[:] if mm_dt == f32 else ident_r[:])
            pT = hpool.tile([128, NC, N], mm_dt)
            nc.scalar.copy(
                pT[:].rearrange("p d (c u) -> p c d u", u=128),
                tp4[:].rearrange("p (c d) u -> p c d u", d=NC))
            # out1T[o, n] = Wh.T @ pT ; elu -> hT rows
            at = hpool.tile([128, NC, N], f32)
            for oc in range(NC):
                ops = ps_big.tile([128, N], f32, tag="big")
                for mc in range(NC):
                    nc.tensor.matmul(
                        ops[:], lhsT=wh_sb[:, mc, oc * 128:(oc + 1) * 128],
                        rhs=pT[:, mc, :], start=(mc == 0), stop=(mc == NC - 1))
                nc.scalar.activation(at[:, oc, :], ops[:], AF.Exp)
                nc.scalar.activation(hT[:, h * NC + oc, :], ops[:], AF.Relu)
            nc.vector.tensor_scalar(
                at[:], at[:], 1.0, 0.0, op0=AL.subtract, op1=AL.min)
            nc.vector.tensor_tensor(
                hT[:, h * NC:(h + 1) * NC, :], at[:],
                hT[:, h * NC:(h + 1) * NC, :], op=AL.add)

        def emit_l2(g):
            s = G[g]
            npm_sb, mneg, hT = s["npm"], s["mneg"], s["hT"]
            wh2_sb = gpool.tile([128, NC, OUT], mm_dt)
            e12 = gpool.tile([128, NC, 2], f32)
            for c in range(NC):
                for half in range(2):
                    hps = ps_aug.tile([128, 258], f32, tag="aug")
                    for k in range(KC2):
                        nc.tensor.matmul(
                            hps[:], lhsT=hT[:, k, c * 128:(c + 1) * 128],
                            rhs=wo_sb[:, k, half, :],
                            start=(k == 0), stop=(k == KC2 - 1))
                    nc.scalar.activation(
                        wh2_sb[:, c, half * 256:(half + 1) * 256],
                        hps[:, 0:256], AF.Copy, scale=npm_sb[:, c:c + 1])
                    nc.scalar.activation(
                        e12[:, c, half:half + 1], hps[:, 256:257], AF.Copy,
                        scale=npm_sb[:, c:c + 1])
            # e2 row via PE transpose of the two column chunks, then broadcast
            e2r_ps = ps_aug.tile([1, N], f32, tag="aug")
            for c in range(NC):
                nc.tensor.transpose(e2r_ps[:, c * 128:(c + 1) * 128],
                                    e12[:, c, 1:2], ident[:])
            e2row2 = gpool.tile([1, N], mm_dt)
            nc.vector.tensor_copy(e2row2[:], e2r_ps[:])
            e2bc2 = ps_bc.tile([128, N], f32, tag="bc")
            nc.tensor.matmul(e2bc2[:], lhsT=ones_sb[:], rhs=e2row2[:],
                             start=True, stop=True)
            em2 = hpool.tile([128, NC, N], f32, tag="em")
            z2t = hpool.tile([128, NC, N], mm_dt, tag="zt")
            z2sum = gpool.tile([128, NC], f32)
            z2inv = gpool.tile([128, NC], f32)
            sc2 = gpool.tile([128, NC], f32)
            bca2 = e2bc2[:]
            e2bc2_b = bass.AP(tensor=bca2.tensor, offset=bca2.offset,
                              ap=[bca2.ap[0], [0, NC], [1, N]])
            nc.vector.tensor_tensor(em2[:], e2bc2_b, mneg[:], op=AL.add)
            for c in range(NC):
                e1col = e12[:, c, 0:1]
                nc.vector.tensor_scalar(
                    z2t[:, c, :], em2[:, c, :], e1col, None, op0=AL.add)
            nc.vector.tensor_scalar_mul(em2[:], z2t[:], ALPHA)
            nc.vector.tensor_tensor(z2t[:], z2t[:], em2[:], op=AL.max)
            p2 = z2t[:]
            for c in range(NC):
                nc.scalar.activation(p2[:, c, :], z2t[:, c, :], AF.Exp,
                                     accum_out=z2sum[:, c:c + 1])
            nc.vector.reciprocal(z2inv[:], z2sum[:])
            nc.vector.tensor_mul(sc2[:], z2inv[:], npm_sb[:])
            tp4b = ps_tr.tile([128, NC * NC, 128], mm_dt, tag="tr")
            for c in range(NC):
                for d in range(NC):
                    nc.tensor.transpose(tp4b[:, c * NC + d, :],
                                        p2[:, c, d * 128:(d + 1) * 128],
                                        ident[:] if mm_dt == f32 else ident_r[:])
            pT2 = hpool.tile([128, NC, N], mm_dt, tag="pT")
            nc.scalar.copy(
                pT2[:].rearrange("p d (c u) -> p c d u", u=128),
                tp4b[:].rearrange("p (c d) u -> p c d u", d=NC))
            out_sb = gpool.tile([128, NC, OUT], f32)
            a2 = gpool.tile([128, NC, OUT], f32)
            for c in range(NC):
                o2ps = ps_big.tile([128, OUT], f32, tag="big")
                for mc in range(NC):
                    nc.tensor.matmul(
                        o2ps[:], lhsT=pT2[:, mc, c * 128:(c + 1) * 128],
                        rhs=wh2_sb[:, mc, :], start=(mc == 0), stop=(mc == NC - 1))
                nc.scalar.activation(a2[:, c, :], o2ps[:], AF.Exp,
                                     scale=sc2[:, c:c + 1])
                nc.scalar.activation(out_sb[:, c, :], o2ps[:], AF.Relu,
                                     scale=sc2[:, c:c + 1])
            nc.vector.tensor_scalar(
                a2[:], a2[:], 1.0, 0.0, op0=AL.subtract, op1=AL.min)
            nc.vector.tensor_tensor(out_sb[:], a2[:], out_sb[:], op=AL.add)
            nc.gpsimd.dma_start(out_d[g], out_sb[:])
            del G[g]

        for gp in range(GPC // 2):
            g0, g1 = 2 * gp, 2 * gp + 1
            emit_setup(g0)
            emit_setup(g1)
            emit_aug(g0, 0)
            emit_aug(g1, 0)
            for h in range(H):
                for g in (g0, g1):
                    if h + 1 < H:
                        emit_aug(g, h + 1)
                    emit_head(g, h)
            emit_l2(g0)
            emit_l2(g1)

    nc.compile()
    return nc


_NC_CACHE = {}


def build_kernel(mm_dt=MM_DT):
    key = str(mm_dt)
    if key not in _NC_CACHE:
        _NC_CACHE[key] = _build_nc(mm_dt)
    return _NC_CACHE[key]


def _host_prep(fea, adj, non_pad_mask, embed, W_heads, a_heads, W_out, a_out):
    """Fold attention vectors into weights (f64) and pre-layout per-core inputs."""
    W64 = W_heads.astype(np.float64)
    w1 = np.einsum("hfo,ho->hf", W64, a_heads[:, :O].astype(np.float64))
    w2 = np.einsum("hfo,ho->hf", W64, a_heads[:, O:].astype(np.float64))
    emb64 = np.zeros((VC * 128, F))
    emb64[:VOCAB] = embed.astype(np.float64)
    # embW[h] = embed @ [W[h] | w1[h] | 0]  -> (256, O+2), exact row-select
    w1aug_full = np.concatenate(
        [W64, w1[:, :, None], np.zeros((H, F, 1))], axis=2)     # (H, F, O+2)
    embw = np.einsum("vf,hfo->hvo", emb64, w1aug_full)          # (H, 256, O+2)
    embw = np.ascontiguousarray(
        embw.reshape(H, VC, 128, O + 2).transpose(2, 0, 1, 3)).astype(np.float32)
    # e2w = embed @ w2.T -> (256, H)
    e2w = np.ascontiguousarray(
        (emb64 @ w2.T).reshape(VC, 128, H).transpose(1, 0, 2)).astype(np.float32)

    Wo64 = W_out.astype(np.float64)
    w1o = Wo64 @ a_out[:OUT].astype(np.float64)
    w2o = Wo64 @ a_out[OUT:].astype(np.float64)
    zcol = np.zeros((H * O, 1))
    woaug = np.concatenate(
        [Wo64[:, 0:256], w1o[:, None], zcol,
         Wo64[:, 256:512], w2o[:, None], zcol], axis=1)       # (2048, 516)
    woaug = np.ascontiguousarray(
        woaug.reshape(KC2, 128, 2, 258).transpose(1, 0, 2, 3)).astype(np.float32)

    vidx = np.arange(VC * 128).reshape(VC, 128)
    oh = (fea[:, None, None, :] == vidx[None, :, :, None])       # (B, VC, 128, N)
    oh = np.ascontiguousarray(oh.transpose(0, 2, 1, 3)).astype(np.float32)

    adjm = np.ascontiguousarray(
        adj.reshape(B, NC, 128, N).transpose(0, 2, 1, 3)).astype(np.int32)
    npm = np.ascontiguousarray(
        non_pad_mask.reshape(B, NC, 128).transpose(0, 2, 1)).astype(np.float32)

    return oh, adjm, npm, embw, e2w, woaug


def kernel(fea, adj, non_pad_mask, embed, W_heads, a_heads, W_out, a_out,
           _mm_dt=None, _trace=False):
    mm_dt = MM_DT if _mm_dt is None else _mm_dt
    oh, adjm, npm, embw, e2w, woaug = _host_prep(
        fea, adj, non_pad_mask, embed, W_heads, a_heads, W_out, a_out)

    nc = build_kernel(mm_dt)
    identity = np.eye(128, dtype=np.float32)
    in_maps = []
    for i in range(NCORES):
        sl = slice(i * GPC, (i + 1) * GPC)
        in_maps.append({
            "oh": oh[sl], "adjm": adjm[sl], "npm": npm[sl],
            "embw": embw, "e2w": e2w, "woaug": woaug,
            "identity": identity,
        })
    res = run_bass_kernel_spmd(nc, in_maps, core_ids=list(range(NCORES)),
                               trace=_trace)
    outs = []
    for i in range(NCORES):
        o = res.results[i]["out"]                   # (GPC, 128, NC, OUT)
        outs.append(o.transpose(0, 2, 1, 3).reshape(GPC, N, OUT))
    full = np.concatenate(outs, axis=0).astype(np.float32)
    if _trace:
        kernel.last_results = res
    return full
